# revision 18
# baseline (speedup 1.0000x reference)
"""Trainium2 Bass kernel for the DVR-JANET recurrent cell.

Strategy: TIME-parallel across cores AND across two dovetailed streams
per core.  The recurrence h' = f*h + (1-f)*g is contractive
(perturbations decay ~0.96x/step, measured), so a chunk of the time
axis recomputed from the zero state converges to the true trajectory:
with W=64 warm-up steps the end-to-end output error is ~4e-3 (measured
with fp16 state rounding), far under the 2e-2 gate.

The 1024 steps split into 16 chunks; core c runs chunk c (stream A) and
chunk c+8 (stream B) simultaneously for all 64 batch rows.  Chunk 0
starts from the true h0, so its whole 124-step window is valid output;
chunks 1..15 contribute their last 60 steps: 124 + 15*60 = 1024.  The
two streams are independent serial chains, emitted half-phase shifted
(A front || B back), so each stream's work fills the other's dependency
bubbles - the per-step wall is chain-latency bound, and two chunks
advance per chain traversal.

Per stream-step (tensors transposed: h on partitions, batch free):
7 HxH weight banks = 28 LDW+MM pairs at FD=64 + 2 init matmuls folding
the rank-1 x-terms and biases into PSUM.  cos(t)=sin(t+pi/2) via ACT
bias; sigmoid via tanh so sin+tanh share one pinned table set; state
history dense (update q=f*h, t=(1-f)*g, h'=q+t split per I/Q half so
the Q-half update overlaps the gc-half matmuls).  Final I/Q projections
are a batched matmul pass over the history."""

import functools
import os
import numpy as np

import concourse.bacc as bacc
import concourse.mybir as mybir
from concourse import tile
import concourse.hw_specs as hw_specs
from concourse.bass_utils import run_bass_kernel_spmd

F32 = mybir.dt.float32
F16 = mybir.dt.float16
AF = mybir.ActivationFunctionType
OP = mybir.AluOpType

B, T, H = 64, 1024, 256
NCORES = 8
NS = 3                    # dovetailed streams per core
NCHUNK = 8 * NS           # time chunks
WARM = 64                 # warm-up steps (discarded) for chunks >= 1
S = 104                   # steps per chunk; S + (NCHUNK-1)*(S-WARM) = 1024
OUTW = S - WARM           # valid output steps for chunks >= 1
OFFS = [OUTW * g for g in range(NCHUNK)]  # chunk start (global t)
CH = 4                    # x-stream chunk length (steps)
PC = 8                    # projection chunk (8 steps * 64 b = 512 psum cols)
HALF_PI = float(np.pi / 2)

# weight tile bank offsets in wt1 (each bank: 4 tiles, idx 2j+k)
TH, A_, F_, GCT, GST, GCB, GSB = 0, 4, 8, 12, 16, 20, 24

# ---------------------------------------------------------------------------
# Pin the ACT table set to silu_and_others (contains sin AND tanh) so the
# compiler never inserts per-step table swaps.  Reload-safe.
_cur = hw_specs.get_activation_tables
_orig_tables = getattr(_cur, "_bass_orig_tables", None) or _cur.__wrapped__


def _pinned_tables(arch):
    full = _orig_tables(arch)
    return {name: (funcs if name == "silu_and_others" else set())
            for name, funcs in full.items()}


def _pin_tables():
    fn = functools.cache(_pinned_tables)
    fn._bass_orig_tables = _orig_tables
    hw_specs.get_activation_tables = fn
    if hasattr(bacc, "get_activation_tables"):
        bacc.get_activation_tables = fn


# ---------------------------------------------------------------------------
_PROG_CACHE = {}


def build_program(Sn=S, sb=0.0, data_S=None, reps=1):
    """Build the 8-core SPMD program.  data_S sizes the DRAM x-stream so
    short-loop timing variants can share input maps with the full build.
    reps>1 repeats the whole computation back-to-back on device (timing:
    wall(reps=R) - wall(reps=1) = (R-1) * true exec span)."""
    if data_S is None:
        data_S = Sn
    key = (Sn, float(sb), data_S, reps)
    if key in _PROG_CACHE:
        return _PROG_CACHE[key]
    _pin_tables()
    nch = (data_S + CH - 1) // CH
    outw = max(0, Sn - WARM) or Sn  # B-stream projected steps
    nc = bacc.Bacc("TRN2", target_bir_lowering=False, debug=False,
                   num_devices=NCORES)

    w1_d = nc.dram_tensor("W1", [128, 28 * 128], F16, kind="ExternalInput").ap()
    xw_d = nc.dram_tensor("XW", [5, 128], F16, kind="ExternalInput").ap()
    xwg_d = nc.dram_tensor("XWG", [6, 128], F16, kind="ExternalInput").ap()
    onesg_d = nc.dram_tensor("ONESG", [6, 384], F16, kind="ExternalInput").ap()
    wp_d = nc.dram_tensor("WP", [128, 4], F16, kind="ExternalInput").ap()
    s0_d = nc.dram_tensor("S0", [128, 256 * NS], F16,
                          kind="ExternalInput").ap()
    xb_d = nc.dram_tensor("XB", [nch, 5, NS * 384 * CH], F16,
                          kind="ExternalInput").ap()
    out_d = nc.dram_tensor("OUT", [2, 64 * (Sn + (NS - 1) * outw)],
                           F16, kind="ExternalOutput").ap()

    with tile.TileContext(
            nc, trace_sim=bool(os.environ.get("KERNEL_TRACE_SIM"))) as tc:
        with (
            tc.tile_pool(name="const", bufs=1) as cpool,
            tc.tile_pool(name="xb", bufs=2) as xbpool,
            tc.tile_pool(name="work0", bufs=2) as wp0,
            tc.tile_pool(name="work1", bufs=2) as wp1,
            tc.tile_pool(name="work2", bufs=2) as wp2,
            tc.tile_pool(name="pab0", bufs=1, space="PSUM") as ppab0,
            tc.tile_pool(name="pg0", bufs=1, space="PSUM") as ppg0,
            tc.tile_pool(name="pab1", bufs=1, space="PSUM") as ppab1,
            tc.tile_pool(name="pg1", bufs=1, space="PSUM") as ppg1,
            tc.tile_pool(name="pab2", bufs=1, space="PSUM") as ppab2,
            tc.tile_pool(name="pg2", bufs=1, space="PSUM") as ppg2,
            tc.tile_pool(name="pproj", bufs=2, space="PSUM") as pproj,
        ):
            wps = [wp0, wp1, wp2]
            ppabs = [ppab0, ppab1, ppab2]
            ppgs = [ppg0, ppg1, ppg2]
            wt1 = cpool.tile([128, 28 * 128], F16, tag="wt1")
            xw = cpool.tile([5, 128], F16, tag="xw")
            xwg = cpool.tile([6, 128], F16, tag="xwg")
            onesg = cpool.tile([6, 384], F16, tag="onesg")
            wp = cpool.tile([128, 4], F16, tag="wp")
            # stream 0: full dense history (slot s = state before step s);
            # streams >= 1: 2 ping-pong warm-up slots + outw+1 output slots
            hist0 = cpool.tile([128, 256 * (Sn + 1)], F16, tag="hist0",
                               name="hist0")
            hists = [hist0]
            for i in range(1, NS):
                hi = cpool.tile([128, 256 * (outw + 3)], F16,
                                tag=f"hist{i}", name=f"hist{i}")
                hists.append(hi)
            iqsI = cpool.tile([1, 64 * (Sn + (NS - 1) * outw)], F16,
                              tag="iqsI")
            iqsQ = cpool.tile([1, 64 * (Sn + (NS - 1) * outw)], F16,
                              tag="iqsQ")
            iqs = [iqsI, iqsQ]

            nc.sync.dma_start(wt1[:], w1_d)
            nc.sync.dma_start(xw[:], xw_d)
            nc.sync.dma_start(xwg[:], xwg_d)
            nc.sync.dma_start(onesg[:], onesg_d)
            nc.sync.dma_start(wp[:], wp_d)

            def wtile(i):
                return wt1[:, 128 * i:128 * (i + 1)]

            def mkslot(i):
                if i == 0:
                    return lambda s: hists[0][:, 256 * s:256 * s + 256]

                def sl(s):
                    if Sn <= WARM:   # short timing builds: all rolling
                        k = s % 2 if s < Sn else 2
                    else:
                        k = s % 2 if s < WARM else 2 + (s - WARM)
                    return hists[i][:, 256 * k:256 * k + 256]
                return sl

            chunk = {"xb": None}

            def h1(st, s):
                """front half: pt, psum inits, th'/th/a/f/top matmuls,
                sin over [th+pi/2 | th], casa."""
                slot = st["slot"](s)
                w = st["wp"]
                pab = st["ppab"].tile([128, 384], F32, tag="pab" + st["nm"])
                pg = st["ppg"].tile([128, 384], F32, tag="pg" + st["nm"])
                pt = w.tile([128, 128], F16, tag="pt")
                psc = w.tile([128, 256], F16, tag="psc")
                casa = w.tile([128, 256], F16, tag="casa")
                tf = w.tile([128, 128], F16, tag="tf")
                st.update(slot_s=slot, pab=pab, pg=pg, psc=psc, casa=casa,
                          tf=tf)

                nc.vector.tensor_mul(pt[:], slot[:, 0:128], slot[:, 128:256])

                xo = NS * 384 * (s % CH) + st["xoff"]
                nc.tensor.matmul(pab[:, 0:384], xw, chunk["xb"][:, xo:xo + 384],
                                 start=True, stop=False)
                nc.tensor.matmul(pg[:, 0:384], xwg, onesg,
                                 start=True, stop=False)
                # th into [128:256], th' (= th + pi/2 via init row) into
                # [0:128]: same weight tiles, same moving operand
                for base, off in ((128, 0), (0, 0)):
                    for j in (0, 1):
                        for k in (0, 1):
                            nc.tensor.matmul(
                                pab[:, base + 64 * j:base + 64 * j + 64],
                                wtile(TH + 2 * j + k),
                                pt[:, 64 * k:64 * k + 64],
                                start=False, stop=(k == 1))
                for j in (0, 1):
                    for k in (0, 1):
                        nc.tensor.matmul(pab[:, 256 + 64 * j:320 + 64 * j],
                                         wtile(A_ + 2 * j + k),
                                         pt[:, 64 * k:64 * k + 64],
                                         start=False, stop=(k == 1))
                for j in (0, 1):
                    for k in (0, 1):
                        nc.tensor.matmul(pg[:, 256 + 64 * j:320 + 64 * j],
                                         wtile(F_ + 2 * j + k),
                                         pt[:, 64 * k:64 * k + 64],
                                         start=False, stop=(k == 1))
                for j in (0, 1):
                    for k in (0, 1):
                        nc.tensor.matmul(pg[:, 64 * j:64 * j + 64],
                                         wtile(GCT + 2 * j + k),
                                         slot[:, 64 * k:64 * k + 64],
                                         start=False, stop=False)
                        nc.tensor.matmul(pg[:, 128 + 64 * j:192 + 64 * j],
                                         wtile(GST + 2 * j + k),
                                         slot[:, 128 + 64 * k:192 + 64 * k],
                                         start=False, stop=False)

                nc.scalar.activation(psc[:], pab[:, 0:256], AF.Sin)
                nc.scalar.activation(tf[:], pg[:, 256:384], AF.Tanh)

                # casa = (a + sb) * [cos | sin]  (a broadcast over halves)
                av = pab[:, 256:384].rearrange("p (o f) -> p o f", o=1) \
                    .broadcast_to([128, 2, 128])
                nc.vector.scalar_tensor_tensor(
                    casa.rearrange("p (o f) -> p o f", o=2), av, float(sb),
                    psc.rearrange("p (o f) -> p o f", o=2), OP.add, OP.mult)

            def h2(st, s):
                """back half: bot matmuls, gate tanh, state update."""
                slot, nslot = st["slot_s"], st["slot"](s + 1)
                pg, casa, tf = st["pg"], st["casa"], st["tf"]
                w = st["wp"]
                fga = w.tile([128, 128], F16, tag="fga")
                dd = w.tile([128, 256], F16, tag="dd")
                ee = w.tile([128, 256], F16, tag="ee")
                gg = w.tile([128, 256], F16, tag="gg")

                for j in (0, 1):
                    for k in (0, 1):
                        nc.tensor.matmul(pg[:, 128 + 64 * j:192 + 64 * j],
                                         wtile(GSB + 2 * j + k),
                                         casa[:, 128 + 64 * k:192 + 64 * k],
                                         start=False, stop=(k == 1))
                for j in (0, 1):
                    for k in (0, 1):
                        nc.tensor.matmul(pg[:, 64 * j:64 * j + 64],
                                         wtile(GCB + 2 * j + k),
                                         casa[:, 64 * k:64 * k + 64],
                                         start=False, stop=(k == 1))
                nc.scalar.activation(gg[:], pg[:, 0:256], AF.Tanh)

                nc.vector.tensor_scalar(fga[:], tf[:], 0.5, 0.5,
                                        OP.mult, OP.add)
                # h' = f*h + (1-f)*g  =  g + f (x) (h - g)
                nc.vector.tensor_sub(dd[:], slot, gg[:])
                fbc = fga.rearrange("p (o f) -> p o f", o=1) \
                         .broadcast_to([128, 2, 128])
                nc.vector.scalar_tensor_tensor(
                    ee.rearrange("p (o f) -> p o f", o=2), fbc, 1.0,
                    dd.rearrange("p (o f) -> p o f", o=2), OP.mult, OP.mult)
                nc.vector.tensor_add(nslot, ee[:], gg[:])

            sts = [{"nm": str(i), "slot": mkslot(i), "wp": wps[i],
                    "ppab": ppabs[i], "ppg": ppgs[i], "xoff": 384 * i}
                   for i in range(NS)]

            def pchunk(i, c, pcnt):
                """projection chunk c of stream i: I and Q rows at once
                (stationary [WI_j | WQ_j], j-accumulated in psum)."""
                nsteps = Sn if i == 0 else outw
                iq0 = 0 if i == 0 else 64 * (Sn + (i - 1) * outw)
                base_slot = 1 if i == 0 else 3
                c0 = PC * c
                tlen = min(PC, nsteps - c0)
                base = 256 * (base_slot + c0)
                rhs3 = hists[i][:, base:base + 256 * tlen] \
                    .rearrange("p (t u) -> p t u", t=tlen)
                for q in (0, 1):
                    pp = pproj.tile([1, 512], F32, tag="pp")
                    for j in (0, 1):
                        u = 2 * q + j
                        nc.tensor.matmul(pp[:, 0:64 * tlen],
                                         wp[:, u:u + 1],
                                         rhs3[:, :, 64 * u:64 * u + 64],
                                         start=(j == 0), stop=(j == 1))
                    dst = iqs[q][0:1, iq0 + 64 * c0:iq0 + 64 * (c0 + tlen)]
                    if (pcnt + q) % 2 == 0:
                        nc.scalar.copy(dst, pp[:, 0:64 * tlen])
                    else:
                        nc.vector.tensor_copy(dst, pp[:, 0:64 * tlen])

            # chunk availability: stream i's proj chunk c readable after
            # step (0 if i==0 else WARM) + PC*c + tlen - 1
            psched = {}
            nproj = []
            for i in range(NS):
                nsteps = Sn if i == 0 else (outw if Sn > WARM else 0)
                w0 = 0 if i == 0 else WARM
                nchk = (nsteps + PC - 1) // PC
                nproj.append(nchk)
                for c in range(nchk):
                    tlen = min(PC, nsteps - PC * c)
                    rdy = w0 + PC * c + tlen   # emit at step index rdy
                    psched.setdefault(rdy, []).append((i, c))

            for rep in range(reps):
                for i in range(NS):
                    nc.sync.dma_start(hists[i][:, 0:256],
                                      s0_d[:, 256 * i:256 * i + 256])
                pcnt = 0
                # dovetail: streams phase-shifted by 2 half-slots each
                for s in range(Sn):
                    if s % CH == 0:
                        chunk["xb"] = xbpool.tile([5, NS * 384 * CH], F16,
                                                  tag="xbt", name="xbt")
                        nc.sync.dma_start(chunk["xb"][:], xb_d[s // CH])
                    h1(sts[0], s)
                    if s > 0:
                        h2(sts[2], s - 1)
                    h1(sts[1], s)
                    for (pi, pc) in psched.get(s, []):
                        pchunk(pi, pc, pcnt)
                        pcnt += 1
                    h2(sts[0], s)
                    h1(sts[2], s)
                    h2(sts[1], s)
                h2(sts[2], Sn - 1)
                for (pi, pc) in psched.get(Sn, []):
                    pchunk(pi, pc, pcnt)
                    pcnt += 1
                for rdy in sorted(k for k in psched if k > Sn):
                    for (pi, pc) in psched[rdy]:
                        pchunk(pi, pc, pcnt)
                        pcnt += 1

                nc.sync.dma_start(out_d[0:1], iqsI[:])
                nc.sync.dma_start(out_d[1:2], iqsQ[:])

    nc.compile()
    _PROG_CACHE[key] = nc
    return nc


# ---------------------------------------------------------------------------
def prepare_inputs(inputs, Sn=S):
    """Host-side preprocessing: weight packing + per-core input maps."""
    f16 = np.float16
    x = np.asarray(inputs["x"], np.float32)
    hI0 = np.asarray(inputs["hI_0"], np.float32)[0]
    hQ0 = np.asarray(inputs["hQ_0"], np.float32)[0]
    c1 = float(np.asarray(inputs["c1"])[0])
    c2 = float(np.asarray(inputs["c2"])[0])
    c3 = float(np.asarray(inputs["c3"])[0])
    sc = c1 + c2 + c3
    sb = -(c1 / 3.0 + 2.0 * c2 / 3.0 + c3)
    Wa = np.asarray(inputs["Wa"], np.float32)[0]
    Wah = np.asarray(inputs["Wah"], np.float32)
    Wp1 = np.asarray(inputs["Wp1"], np.float32)[0]
    Wph = np.asarray(inputs["Wph"], np.float32)
    Wf = np.asarray(inputs["Wf"], np.float32)
    bf = np.asarray(inputs["bf"], np.float32)
    Wgc = np.asarray(inputs["Wgc"], np.float32)
    bgc = np.asarray(inputs["bgc"], np.float32)
    Wgs = np.asarray(inputs["Wgs"], np.float32)
    bgs = np.asarray(inputs["bgs"], np.float32)
    WI = np.asarray(inputs["WI"], np.float32)
    WQ = np.asarray(inputs["WQ"], np.float32)

    def tiles4(W):
        return [W[128 * k:128 * (k + 1), 128 * j:128 * (j + 1)]
                for j in (0, 1) for k in (0, 1)]

    tl = []
    tl += tiles4(Wph)               # TH
    tl += tiles4(sc * Wah)          # A_ (pre-scaled)
    tl += tiles4(0.5 * Wf)          # F_
    tl += tiles4(Wgc[:H])           # GCT
    tl += tiles4(Wgs[:H])           # GST
    tl += tiles4(Wgc[H:])           # GCB
    tl += tiles4(Wgs[H:])           # GSB
    W1 = np.concatenate(tl, axis=1).astype(f16)

    XW = np.stack([Wp1[0:128], Wp1[128:256],
                   (sc * Wa)[0:128], (sc * Wa)[128:256],
                   np.full(128, np.pi / 2, np.float32)]).astype(f16)
    XWG = np.stack([bgc[0:128], bgc[128:256], bgs[0:128], bgs[128:256],
                    0.5 * bf[0:128], 0.5 * bf[128:256]]).astype(f16)
    ONESG = np.zeros((6, 384), np.float32)
    for i in range(6):
        ONESG[i, 64 * i:64 * i + 64] = 1.0
    ONESG = ONESG.astype(f16)
    WP = np.stack([WI[0:128], WI[128:256], WQ[0:128], WQ[128:256]],
                  axis=1).astype(f16)

    nch = (Sn + CH - 1) // CH
    npad = nch * CH

    def xblock(t0):
        x1p = np.zeros((npad, B), np.float32)
        x0p = np.zeros((npad, B), np.float32)
        tend = min(T, t0 + npad)
        if tend > t0:
            x1p[:tend - t0] = x[:, t0:tend, 1].T
            x0p[:tend - t0] = x[:, t0:tend, 0].T
        blk = np.zeros((npad, 5, 384), np.float32)
        blk[:, 0, 0:64] = x1p        # th' j0
        blk[:, 0, 128:192] = x1p     # th  j0
        blk[:, 1, 64:128] = x1p      # th' j1
        blk[:, 1, 192:256] = x1p     # th  j1
        blk[:, 2, 256:320] = x0p     # a j0
        blk[:, 3, 320:384] = x0p     # a j1
        blk[:, 4, 0:128] = 1.0       # +pi/2 on th' blocks
        return blk

    in_maps = []
    for c in range(NCORES):
        XB = np.concatenate([xblock(OFFS[c + 8 * i]) for i in range(NS)],
                            axis=2)
        XB = XB.reshape(nch, CH, 5, NS * 384).transpose(0, 2, 1, 3)
        S0 = np.zeros((128, 256 * NS), np.float32)
        if c == 0:
            for j in (0, 1):
                S0[:, 64 * j:64 * j + 64] = hI0[:, 128 * j:128 * (j + 1)].T
                S0[:, 128 + 64 * j:192 + 64 * j] = \
                    hQ0[:, 128 * j:128 * (j + 1)].T
        in_maps.append({
            "W1": W1, "XW": XW, "XWG": XWG, "ONESG": ONESG, "WP": WP,
            "S0": S0.astype(f16),
            "XB": np.ascontiguousarray(XB).reshape(nch, 5, NS * 384 * CH)
                    .astype(f16),
        })
    return in_maps, sb


def assemble(results, inputs, Sn=S):
    bI = float(np.asarray(inputs["bI"])[0])
    bQ = float(np.asarray(inputs["bQ"])[0])
    outw = max(0, Sn - WARM) or Sn
    out = np.zeros((B, T, 2), np.float32)
    for c in range(NCORES):
        v = results[c]["OUT"].astype(np.float32)     # [2, cols]
        vA = v[:, :64 * Sn].reshape(2, Sn, 64)
        s0 = 0 if c == 0 else WARM
        for s in range(s0, Sn):
            t = OFFS[c] + s
            out[:, t, 0] = vA[0, s] + bI
            out[:, t, 1] = vA[1, s] + bQ
        for i in range(1, NS):
            o0 = 64 * (Sn + (i - 1) * outw)
            vB = v[:, o0:o0 + 64 * outw].reshape(2, outw, 64)
            for k in range(outw):
                t = OFFS[c + 8 * i] + WARM + k
                out[:, t, 0] = vB[0, k] + bI
                out[:, t, 1] = vB[1, k] + bQ
    return out


def kernel(**inputs) -> np.ndarray:
    in_maps, sb = prepare_inputs(inputs, S)
    nc = build_program(S, sb)
    res = run_bass_kernel_spmd(nc, in_maps, list(range(NCORES)))
    return assemble(res.results, inputs, S)
